# revision 1
# baseline (speedup 1.0000x reference)
"""Causal multi-head self-attention (B=2, S=2048, H=16, hd=128) on 8 trn2 cores.

Tensor-parallel over heads: each core computes 2 heads end-to-end
(QKV projections, causal softmax attention, its slice of the output
projection) and writes a partial output; the host sums the 8 partials.

Dataflow (per core, fp32r matmuls, fp32 PSUM):
  - host supplies x^T [D, B*S] so Q^T/K^T land with head_dim on partitions
  - scores are computed TRANSPOSED (s_kv on partitions, t_q on free axis):
      scoresT = kT_blk.T @ qT  -> exp on ACT -> masked on DVE
  - softmax denominators via ones-vector matmul (column sums in PSUM)
  - attn_outT = v_blk.T(natural v) @ expPT  (no on-chip transposes anywhere)
  - output projection consumes attn_outT as lhsT; the 1/rowsum softmax
    normalization is applied at PSUM eviction as a per-partition scale.

Measured (axon trn2, single-core program == per-core SPMD time):
  HW exec ~330 us/core, relative error vs fp32 reference ~3.6e-4.
  Phase split (HW-measured): projections ~163us (PE 109 + fp32r in-matmul
  weight loads), attention ~85us (at PE roofline), output proj ~100us
  (bound by the 33.5MB fp32 partial-output DMA).
"""

import math

import numpy as np

import concourse.bass as bass
import concourse.mybir as mybir
import concourse.tile as tile
from concourse import bacc
from concourse.bass_utils import run_bass_kernel_spmd

B, S, H, HD = 2, 2048, 16, 128
D = H * HD            # 2048
T = B * S             # 4096
NCORES = 8
HC = H // NCORES      # heads per core: 2
DC = HC * HD          # per-core head dims: 256
KB = D // 128         # contraction k-blocks: 16
PCH = 256             # projection t-chunk
ACH = 512             # attention t-chunk
SB = S // 128         # s blocks per batch: 16
AJ = S // ACH         # attention chunks per batch: 4

dt = mybir.dt
Alu = mybir.AluOpType
Act = mybir.ActivationFunctionType

_CACHE: dict = {}


def round_fp32r(x: np.ndarray) -> np.ndarray:
    """Round fp32 to the fp32r grid (11 explicit mantissa bits, RNE)."""
    u = np.ascontiguousarray(x, dtype=np.float32).view(np.uint32)
    lsb = (u >> np.uint32(12)) & np.uint32(1)
    u = u + np.uint32(0x7FF) + lsb
    u &= np.uint32(0xFFFFF000)
    return u.view(np.float32)


def build_nc(reps: int = 1, pch: int = PCH, expp_bufs: int = 4,
             osb_bufs: int = 2, xpan_bufs: int = 3, interleave_h: bool = False,
             phases: str = "abc", out_fp16: bool = False,
             av_first: bool = True, acc_bufs: int = 3, sc_bufs: int = 2):
    nc = bacc.Bacc("TRN2", target_bir_lowering=False, debug=False,
                   num_devices=NCORES, enable_asserts=False)

    xT = nc.dram_tensor("xT", [D, T], dt.float32r, kind="ExternalInput")
    wq = nc.dram_tensor("wq", [D, DC], dt.float32r, kind="ExternalInput")
    wk = nc.dram_tensor("wk", [D, DC], dt.float32r, kind="ExternalInput")
    wv = nc.dram_tensor("wv", [D, DC], dt.float32r, kind="ExternalInput")
    wo = nc.dram_tensor("wo", [DC, D], dt.float32r, kind="ExternalInput")
    bq = nc.dram_tensor("bq", [HC, 128], dt.float32, kind="ExternalInput")
    bk = nc.dram_tensor("bk", [HC, 128], dt.float32, kind="ExternalInput")
    bv = nc.dram_tensor("bv", [1, DC], dt.float32, kind="ExternalInput")
    mask = nc.dram_tensor("mask", [128, 128], dt.float32r, kind="ExternalInput")
    ones = nc.dram_tensor("ones", [128, 1], dt.float32r, kind="ExternalInput")
    out_dt = dt.float16 if out_fp16 else dt.float32
    out = nc.dram_tensor("out", [T, D], out_dt, kind="ExternalOutput")
    dbg = nc.dram_tensor("dbg", [128, 64], dt.float32, kind="ExternalOutput")

    with tile.TileContext(nc) as tc:
        with tc.tile_pool(name="consts", bufs=1) as consts, \
             tc.tile_pool(name="xpan", bufs=xpan_bufs) as xpan_pool, \
             tc.tile_pool(name="qkv", bufs=1) as qkv_pool, \
             tc.tile_pool(name="expp", bufs=expp_bufs) as expp_pool, \
             tc.tile_pool(name="aout", bufs=1) as aout_pool, \
             tc.tile_pool(name="osb", bufs=osb_bufs) as osb_pool, \
             tc.tile_pool(name="small", bufs=1) as small_pool, \
             tc.tile_pool(name="dram", bufs=1, space="DRAM") as dram_pool, \
             tc.tile_pool(name="ps", bufs=1, space="PSUM") as ps_pool:

            # ---- constants ----
            wq_t, wk_t, wv_t = [], [], []
            for name, src, lst in [("wq", wq, wq_t), ("wk", wk, wk_t),
                                   ("wv", wv, wv_t)]:
                r = src.rearrange("(kb p) m -> kb p m", p=128)
                for kb in range(KB):
                    t_ = consts.tile([128, DC], dt.float32r, tag=f"{name}{kb}")
                    nc.sync.dma_start(out=t_, in_=r[kb])
                    lst.append(t_)
            wo_t = []
            wor = wo.rearrange("(h p) n -> h p n", p=128)
            for h in range(HC):
                t_ = consts.tile([128, D], dt.float32r, tag=f"wo{h}")
                nc.sync.dma_start(out=t_, in_=wor[h])
                wo_t.append(t_)
            bq_t = consts.tile([128, HC], dt.float32, tag="bq")
            nc.sync.dma_start(out=bq_t, in_=bq.rearrange("m p -> p m"))
            bk_t = consts.tile([128, HC], dt.float32, tag="bk")
            nc.sync.dma_start(out=bk_t, in_=bk.rearrange("m p -> p m"))
            bv_t = consts.tile([128, DC], dt.float32, tag="bv")
            nc.sync.dma_start(out=bv_t, in_=bv.ap().broadcast_to((128, DC)))
            mask_t = consts.tile([128, 128], dt.float32r, tag="mask")
            nc.sync.dma_start(out=mask_t, in_=mask.ap())
            ones_t = consts.tile([128, 1], dt.float32r, tag="ones")
            nc.sync.dma_start(out=ones_t, in_=ones.ap())

            for rep in range(reps):
                for b in range(B):
                    # ================= PHASE A: projections =================
                    qT = [qkv_pool.tile([128, S], dt.float32r, tag=f"q{h}", name=f"qT{h}")
                          for h in range(HC)]
                    kT = [qkv_pool.tile([128, S], dt.float32r, tag=f"k{h}", name=f"kT{h}")
                          for h in range(HC)]
                    vt = [qkv_pool.tile([128, DC], dt.float32r, tag=f"v{sb}", name=f"vt{sb}")
                          for sb in range(SB)]
                    for tch in range(S // pch):
                        t0 = b * S + tch * pch
                        xr = xT.rearrange("(kb p) t -> kb p t", p=128)
                        xpan = []
                        for kb in range(KB):
                            xp = xpan_pool.tile([128, pch], dt.float32r,
                                                tag=f"x{kb}")
                            nc.sync.dma_start(out=xp,
                                              in_=xr[kb][:, t0:t0 + pch])
                            xpan.append(xp)
                        cols = slice(tch * pch, (tch + 1) * pch)
                        for proj, wt, bt, dst in [(0, wq_t, bq_t, qT),
                                                  (1, wk_t, bk_t, kT)]:
                            for mb in range(HC):
                                ps = ps_pool.tile([128, pch], dt.float32,
                                                  tag="acc", bufs=acc_bufs)
                                for kb in range(KB):
                                    nc.tensor.matmul(
                                        ps, wt[kb][:, mb * 128:(mb + 1) * 128],
                                        xpan[kb], start=(kb == 0),
                                        stop=(kb == KB - 1))
                                nc.vector.tensor_scalar_add(
                                    dst[mb][:, cols], ps, bt[:, mb:mb + 1])
                        for tm in range(pch // 128):
                            sb = (tch * pch) // 128 + tm
                            ps = ps_pool.tile([128, DC], dt.float32, tag="acc", bufs=acc_bufs)
                            for kb in range(KB):
                                nc.tensor.matmul(
                                    ps, xpan[kb][:, tm * 128:(tm + 1) * 128],
                                    wv_t[kb], start=(kb == 0),
                                    stop=(kb == KB - 1))
                            nc.vector.tensor_add(vt[sb], ps, bv_t)

                    if "b" not in phases:
                        for h in range(HC):
                            nc.sync.dma_start(out=dbg.ap()[:, h:h + 1],
                                              in_=qT[h][:, 0:1].bitcast(dt.float32))
                            nc.sync.dma_start(out=dbg.ap()[:, 2 + h:3 + h],
                                              in_=kT[h][:, 0:1].bitcast(dt.float32))
                        for sb_ in range(SB):
                            nc.sync.dma_start(out=dbg.ap()[:, 8 + sb_:9 + sb_],
                                              in_=vt[sb_][:, 0:1].bitcast(dt.float32))
                        continue
                    # ================= PHASE B: attention =================
                    aout, invT = [], []
                    ao_l = [aout_pool.tile([128, S], dt.float32r, tag=f"a{h}",
                                           name=f"ao{h}") for h in range(HC)]
                    csT_l = [small_pool.tile([128, SB], dt.float32,
                                             tag=f"cs{h}", name=f"csT{h}")
                             for h in range(HC)]
                    scr_l = [dram_pool.tile([SB, 128], dt.float32,
                                            tag=f"scr{h}", name=f"scr{h}")
                             for h in range(HC)]
                    if interleave_h:
                        hj_order = [(h, j) for j in range(AJ)
                                    for h in range(HC)]
                    else:
                        hj_order = [(h, j) for h in range(HC)
                                    for j in range(AJ)]
                    for h, j in hj_order:
                        ao, csT, scr = ao_l[h], csT_l[h], scr_l[h]
                        hc = slice(h * 128, (h + 1) * 128)
                        if True:
                            nblk = 4 * j + 4
                            tj = slice(j * ACH, (j + 1) * ACH)
                            expp = []
                            for i in range(nblk):
                                off = max(0, 128 * i - ACH * j)
                                ps_s = ps_pool.tile([128, ACH], dt.float32,
                                                    tag="sc", bufs=sc_bufs)
                                nc.tensor.matmul(
                                    ps_s[:, off:ACH],
                                    kT[h][:, i * 128:(i + 1) * 128],
                                    qT[h][:, j * ACH + off:(j + 1) * ACH],
                                    start=True, stop=True)
                                ep = expp_pool.tile([128, ACH], dt.float32r,
                                                    tag="e")
                                nc.scalar.activation(ep[:, off:ACH],
                                                     ps_s[:, off:ACH], Act.Exp)
                                if 128 * i >= ACH * j:  # diagonal block
                                    nc.vector.tensor_mul(
                                        ep[:, off:off + 128],
                                        ep[:, off:off + 128], mask_t)
                                expp.append(ep)
                            cs_ps = ps_pool.tile([1, ACH], dt.float32,
                                                 tag="cs", bufs=1)
                            av_ps = ps_pool.tile([128, ACH], dt.float32,
                                                 tag="acc", bufs=acc_bufs)
                            for i in range(nblk):
                                off = max(0, 128 * i - ACH * j)
                                if av_first:
                                    nc.tensor.matmul(av_ps[:, off:ACH],
                                                     vt[i][:, hc],
                                                     expp[i][:, off:ACH],
                                                     start=(i == 0),
                                                     stop=(i == nblk - 1))
                                    nc.tensor.matmul(cs_ps[0:1, off:ACH],
                                                     ones_t,
                                                     expp[i][:, off:ACH],
                                                     start=(i == 0),
                                                     stop=(i == nblk - 1))
                                else:
                                    nc.tensor.matmul(cs_ps[0:1, off:ACH],
                                                     ones_t,
                                                     expp[i][:, off:ACH],
                                                     start=(i == 0),
                                                     stop=(i == nblk - 1))
                                    nc.tensor.matmul(av_ps[:, off:ACH],
                                                     vt[i][:, hc],
                                                     expp[i][:, off:ACH],
                                                     start=(i == 0),
                                                     stop=(i == nblk - 1))
                            nc.vector.tensor_copy(ao[:, tj], av_ps)
                            cs_sb = small_pool.tile([1, ACH], dt.float32,
                                                    tag="cssb")
                            nc.vector.tensor_copy(cs_sb, cs_ps)
                            nc.sync.dma_start(
                                out=scr.rearrange("t p -> (t p)").unsqueeze(0)
                                [0:1, j * ACH:(j + 1) * ACH],
                                in_=cs_sb)
                    for h in range(HC):
                        nc.sync.dma_start(out=csT_l[h],
                                          in_=scr_l[h].rearrange("t p -> p t"))
                        iv = small_pool.tile([128, SB], dt.float32,
                                             tag=f"iv{h}", name=f"iv{h}")
                        nc.vector.reciprocal(iv, csT_l[h])
                        aout.append(ao_l[h])
                        invT.append(iv)

                    if "c" not in phases:
                        for h in range(HC):
                            nc.sync.dma_start(out=dbg.ap()[:, 32 + h:33 + h],
                                              in_=aout[h][:, 0:1].bitcast(dt.float32))
                            nc.sync.dma_start(
                                out=dbg.ap()[:, 34 + h:35 + h],
                                in_=invT[h][:, 0:1])
                        continue
                    # ================= PHASE C: output projection ==========
                    for tb in range(SB):
                        tr = slice(tb * 128, (tb + 1) * 128)
                        for nch in range(D // 512):
                            nr = slice(nch * 512, (nch + 1) * 512)
                            ps0 = ps_pool.tile([128, 512], dt.float32, tag="o", bufs=2)
                            ps1 = ps_pool.tile([128, 512], dt.float32, tag="o", bufs=2)
                            nc.tensor.matmul(ps0, aout[0][:, tr],
                                             wo_t[0][:, nr],
                                             start=True, stop=True)
                            nc.tensor.matmul(ps1, aout[1][:, tr],
                                             wo_t[1][:, nr],
                                             start=True, stop=True)
                            tmp = osb_pool.tile([128, 512], dt.float32,
                                                tag="tmp")
                            nc.scalar.activation(tmp, ps0, Act.Copy,
                                                 scale=invT[0][:, tb:tb + 1])
                            fin = osb_pool.tile([128, 512], out_dt,
                                                tag="fin")
                            nc.vector.scalar_tensor_tensor(
                                fin, ps1, invT[1][:, tb:tb + 1], tmp,
                                op0=Alu.mult, op1=Alu.add)
                            nc.sync.dma_start(
                                out=out.ap()[b * S + tb * 128:
                                             b * S + (tb + 1) * 128, nr],
                                in_=fin)

    nc.compile()
    return nc


def make_in_maps(hidden_states, Wq, bq, Wk, bk, Wv, bv, Wo, bo):
    x2 = np.asarray(hidden_states, dtype=np.float32).reshape(T, D)
    xTr = round_fp32r(x2.T)
    scale = 1.0 / math.sqrt(HD)
    mask_np = np.triu(np.ones((128, 128), dtype=np.float32))
    ones_np = np.ones((128, 1), dtype=np.float32)
    in_maps = []
    for c in range(NCORES):
        sl = slice(c * DC, (c + 1) * DC)
        in_maps.append({
            "xT": xTr,
            "wq": round_fp32r((np.asarray(Wq)[sl] * scale).T),
            "wk": round_fp32r(np.asarray(Wk)[sl].T),
            "wv": round_fp32r(np.asarray(Wv)[sl].T),
            "wo": round_fp32r(np.asarray(Wo)[:, sl].T),
            "bq": np.ascontiguousarray(
                (np.asarray(bq)[sl] * scale).reshape(HC, 128)),
            "bk": np.ascontiguousarray(
                np.asarray(bk)[sl].reshape(HC, 128).astype(np.float32)),
            "bv": np.ascontiguousarray(
                np.asarray(bv)[sl].reshape(1, DC).astype(np.float32)),
            "mask": mask_np,
            "ones": ones_np,
        })
    return in_maps


def kernel(hidden_states, Wq, bq, Wk, bk, Wv, bv, Wo, bo):
    if "nc" not in _CACHE:
        _CACHE["nc"] = build_nc()
    nc = _CACHE["nc"]
    in_maps = make_in_maps(hidden_states, Wq, bq, Wk, bk, Wv, bv, Wo, bo)
    res = run_bass_kernel_spmd(nc, in_maps, list(range(NCORES)))
    acc = np.zeros((T, D), dtype=np.float64)
    for c in range(NCORES):
        acc += res.results[c]["out"].astype(np.float64)
    acc += np.asarray(bo, dtype=np.float64)
    return acc.astype(np.float32).reshape(B, S, D)



# revision 4
# speedup vs baseline: 1595.8652x; 1595.8652x over previous
"""Causal multi-head self-attention (B=2, S=2048, H=16, hd=128) on 8 trn2 cores.

Tensor-parallel over heads: each core computes 2 heads end-to-end
(QKV projections, causal softmax attention, its slice of the output
projection) and writes a partial output; the host sums the 8 partials.

All matmuls run in fp16 (fp32 PSUM accumulate): same 1 row/cycle PE
streaming as fp32r but with FWL weight loads, no sub-256 free-dim
penalty, and half the DMA traffic. Final rel err vs the fp32
reference is ~7e-4 (tolerance 2e-2).

Dataflow (per core):
  - host supplies x^T [D, B*S] fp16 so Q^T/K^T land with head_dim on
    partitions; scores are computed TRANSPOSED (s_kv on partitions,
    t_q on free): scoresT = kT_blk.T @ qT -> exp on ACT -> mask on DVE
  - softmax denominators: DVE accumulates exp blocks into tot[128,ACH]
    (fp16, 2x mode), then ONE ones-vector matmul per (head, q-chunk)
    gives d[1, ACH]; reciprocal (DVE) is broadcast-DMA'd to [128, ACH]
    and folded into the attention-output PSUM eviction (DVE mul), so
    aout is pre-normalized.
  - output projection accumulates both heads into one PSUM bank and
    evicts with a plain copy, alternating ACT/DVE; fp16 partial out.
"""

import math

import numpy as np

import concourse.bass as bass
import concourse.mybir as mybir
import concourse.tile as tile
from concourse import bacc
from concourse.bass_utils import run_bass_kernel_spmd

B, S, H, HD = 2, 2048, 16, 128
D = H * HD            # 2048
T = B * S             # 4096
NCORES = 8
HC = H // NCORES      # heads per core: 2
DC = HC * HD          # per-core head dims: 256
KB = D // 128         # contraction k-blocks: 16
PCH = 256             # projection t-chunk
ACH = 512             # attention t-chunk
SB = S // 128         # s blocks per batch: 16
AJ = S // ACH         # attention chunks per batch: 4

dt = mybir.dt
Alu = mybir.AluOpType
Act = mybir.ActivationFunctionType

_CACHE: dict = {}


def build_nc(reps: int = 1, pch: int = PCH, expp_bufs: int = 10,
             osb_bufs: int = 3, xpan_bufs: int = 3,
             phases: str = "abc", acc_bufs: int = 3, sc_bufs: int = 2):
    nc = bacc.Bacc("TRN2", target_bir_lowering=False, debug=False,
                   num_devices=NCORES, enable_asserts=False)

    mdt = dt.float16
    xT = nc.dram_tensor("xT", [D, T], mdt, kind="ExternalInput")
    wq = nc.dram_tensor("wq", [D, DC], mdt, kind="ExternalInput")
    wk = nc.dram_tensor("wk", [D, DC], mdt, kind="ExternalInput")
    wv = nc.dram_tensor("wv", [D, DC], mdt, kind="ExternalInput")
    wo = nc.dram_tensor("wo", [DC, D], mdt, kind="ExternalInput")
    bq = nc.dram_tensor("bq", [HC, 128], dt.float32, kind="ExternalInput")
    bk = nc.dram_tensor("bk", [HC, 128], dt.float32, kind="ExternalInput")
    bv = nc.dram_tensor("bv", [1, DC], dt.float32, kind="ExternalInput")
    mask = nc.dram_tensor("mask", [128, 128], mdt, kind="ExternalInput")
    ones = nc.dram_tensor("ones", [128, 1], mdt, kind="ExternalInput")
    out = nc.dram_tensor("out", [T, D], dt.float16, kind="ExternalOutput")
    dbg = nc.dram_tensor("dbg", [128, 64], dt.float32, kind="ExternalOutput")

    with tile.TileContext(nc) as tc:
        with tc.tile_pool(name="consts", bufs=1) as consts, \
             tc.tile_pool(name="xpan", bufs=xpan_bufs) as xpan_pool, \
             tc.tile_pool(name="qkv", bufs=1) as qkv_pool, \
             tc.tile_pool(name="expp", bufs=expp_bufs) as expp_pool, \
             tc.tile_pool(name="tot", bufs=2) as tot_pool, \
             tc.tile_pool(name="invb", bufs=2) as invb_pool, \
             tc.tile_pool(name="aout", bufs=1) as aout_pool, \
             tc.tile_pool(name="osb", bufs=osb_bufs) as osb_pool, \
             tc.tile_pool(name="small", bufs=1) as small_pool, \
             tc.tile_pool(name="dram", bufs=1, space="DRAM") as dram_pool, \
             tc.tile_pool(name="ps", bufs=1, space="PSUM") as ps_pool:

            # ---- constants ----
            wq_t, wk_t, wv_t = [], [], []
            for name, src, lst in [("wq", wq, wq_t), ("wk", wk, wk_t),
                                   ("wv", wv, wv_t)]:
                r = src.rearrange("(kb p) m -> kb p m", p=128)
                for kb in range(KB):
                    t_ = consts.tile([128, DC], mdt, tag=f"{name}{kb}")
                    nc.sync.dma_start(out=t_, in_=r[kb])
                    lst.append(t_)
            wo_t = []
            wor = wo.rearrange("(h p) n -> h p n", p=128)
            for h in range(HC):
                t_ = consts.tile([128, D], mdt, tag=f"wo{h}")
                nc.sync.dma_start(out=t_, in_=wor[h])
                wo_t.append(t_)
            bq_t = consts.tile([128, HC], dt.float32, tag="bq")
            nc.sync.dma_start(out=bq_t, in_=bq.rearrange("m p -> p m"))
            bk_t = consts.tile([128, HC], dt.float32, tag="bk")
            nc.sync.dma_start(out=bk_t, in_=bk.rearrange("m p -> p m"))
            bv_t = consts.tile([128, DC], dt.float32, tag="bv")
            nc.sync.dma_start(out=bv_t, in_=bv.ap().broadcast_to((128, DC)))
            mask_t = consts.tile([128, 128], mdt, tag="mask")
            nc.sync.dma_start(out=mask_t, in_=mask.ap())
            ones_t = consts.tile([128, 1], mdt, tag="ones")
            nc.sync.dma_start(out=ones_t, in_=ones.ap())

            for rep in range(reps):
                for b in range(B):
                    # ================= PHASE A: projections =================
                    qT = [qkv_pool.tile([128, S], mdt, tag=f"q{h}", name=f"qT{h}")
                          for h in range(HC)]
                    kT = [qkv_pool.tile([128, S], mdt, tag=f"k{h}", name=f"kT{h}")
                          for h in range(HC)]
                    vt = [qkv_pool.tile([128, DC], mdt, tag=f"v{sb}", name=f"vt{sb}")
                          for sb in range(SB)]
                    for tch in range(S // pch):
                        t0 = b * S + tch * pch
                        xr = xT.rearrange("(kb p) t -> kb p t", p=128)
                        xpan = []
                        for kb in range(KB):
                            xp = xpan_pool.tile([128, pch], mdt, tag=f"x{kb}")
                            nc.sync.dma_start(out=xp,
                                              in_=xr[kb][:, t0:t0 + pch])
                            xpan.append(xp)
                        cols = slice(tch * pch, (tch + 1) * pch)
                        for proj, wt, bt, dst in [(0, wq_t, bq_t, qT),
                                                  (1, wk_t, bk_t, kT)]:
                            for mb in range(HC):
                                ps = ps_pool.tile([128, pch], dt.float32,
                                                  tag="acc", bufs=acc_bufs)
                                for kb in range(KB):
                                    nc.tensor.matmul(
                                        ps, wt[kb][:, mb * 128:(mb + 1) * 128],
                                        xpan[kb], start=(kb == 0),
                                        stop=(kb == KB - 1))
                                nc.vector.tensor_scalar_add(
                                    dst[mb][:, cols], ps, bt[:, mb:mb + 1])
                        for tm in range(pch // 128):
                            sb = (tch * pch) // 128 + tm
                            ps = ps_pool.tile([128, DC], dt.float32, tag="acc",
                                              bufs=acc_bufs)
                            for kb in range(KB):
                                nc.tensor.matmul(
                                    ps, xpan[kb][:, tm * 128:(tm + 1) * 128],
                                    wv_t[kb], start=(kb == 0),
                                    stop=(kb == KB - 1))
                            nc.vector.tensor_add(vt[sb], ps, bv_t)

                    if "b" not in phases:
                        for h in range(HC):
                            nc.sync.dma_start(out=dbg.ap()[:, h:h + 1],
                                              in_=qT[h][:, 0:1].bitcast(dt.float32))
                            nc.sync.dma_start(out=dbg.ap()[:, 2 + h:3 + h],
                                              in_=kT[h][:, 0:1].bitcast(dt.float32))
                        for sb_ in range(SB):
                            nc.sync.dma_start(out=dbg.ap()[:, 8 + sb_:9 + sb_],
                                              in_=vt[sb_][:, 0:1].bitcast(dt.float32))
                        continue
                    # ================= PHASE B: attention =================
                    ao_l = [aout_pool.tile([128, S], mdt, tag=f"a{h}",
                                           name=f"ao{h}") for h in range(HC)]

                    def emit_cs(h, j, tot, av_ps):
                        """Denominator + normalized eviction for chunk (h, j).
                        Called one chunk late so the PE never waits on tot."""
                        tj = slice(j * ACH, (j + 1) * ACH)
                        cs_ps = ps_pool.tile([1, ACH], dt.float32,
                                             tag="cs", bufs=1)
                        nc.tensor.matmul(cs_ps, ones_t, tot,
                                         start=True, stop=True)
                        csi = small_pool.tile([1, ACH], dt.float32,
                                              tag="csi", bufs=2)
                        nc.vector.reciprocal(csi, cs_ps)
                        csd = dram_pool.tile([1, ACH], dt.float32,
                                             tag="csd", bufs=2)
                        nc.sync.dma_start(out=csd, in_=csi)
                        invb = invb_pool.tile([128, ACH], dt.float32,
                                              tag="invb")
                        nc.sync.dma_start(
                            out=invb,
                            in_=csd[0:1, :].broadcast_to((128, ACH)))
                        nc.vector.tensor_mul(ao_l[h][:, tj], av_ps, invb)

                    pending = None
                    for h in range(HC):
                        for j in range(AJ):
                            hc = slice(h * 128, (h + 1) * 128)
                            nblk = 4 * j + 4
                            tot = tot_pool.tile([128, ACH], mdt, tag="tot")
                            expp = []
                            for i in range(nblk):
                                off = max(0, 128 * i - ACH * j)
                                ps_s = ps_pool.tile([128, ACH], dt.float32,
                                                    tag="sc", bufs=sc_bufs)
                                nc.tensor.matmul(
                                    ps_s[:, off:ACH],
                                    kT[h][:, i * 128:(i + 1) * 128],
                                    qT[h][:, j * ACH + off:(j + 1) * ACH],
                                    start=True, stop=True)
                                ep = expp_pool.tile([128, ACH], mdt, tag="e")
                                nc.scalar.activation(ep[:, off:ACH],
                                                     ps_s[:, off:ACH], Act.Exp)
                                if 128 * i >= ACH * j:  # diagonal block
                                    nc.vector.tensor_mul(
                                        ep[:, off:off + 128],
                                        ep[:, off:off + 128], mask_t)
                                if i == 0:
                                    nc.vector.tensor_copy(tot, ep)
                                else:
                                    nc.vector.tensor_add(
                                        tot[:, off:ACH], tot[:, off:ACH],
                                        ep[:, off:ACH])
                                expp.append(ep)
                            if pending is not None:
                                emit_cs(*pending)
                                pending = None
                            av_ps = ps_pool.tile([128, ACH], dt.float32,
                                                 tag="acc", bufs=acc_bufs)
                            for i in range(nblk):
                                off = max(0, 128 * i - ACH * j)
                                nc.tensor.matmul(av_ps[:, off:ACH],
                                                 vt[i][:, hc],
                                                 expp[i][:, off:ACH],
                                                 start=(i == 0),
                                                 stop=(i == nblk - 1))
                            pending = (h, j, tot, av_ps)
                    emit_cs(*pending)
                    pending = None

                    if "c" not in phases:
                        for h in range(HC):
                            nc.sync.dma_start(out=dbg.ap()[:, 32 + h:33 + h],
                                              in_=ao_l[h][:, 0:1].bitcast(dt.float32))
                        continue
                    # ================= PHASE C: output projection ==========
                    for tb in range(SB):
                        tr = slice(tb * 128, (tb + 1) * 128)
                        for nch in range(D // 512):
                            nr = slice(nch * 512, (nch + 1) * 512)
                            ps = ps_pool.tile([128, 512], dt.float32,
                                              tag="o", bufs=2)
                            nc.tensor.matmul(ps, ao_l[0][:, tr],
                                             wo_t[0][:, nr],
                                             start=True, stop=False)
                            nc.tensor.matmul(ps, ao_l[1][:, tr],
                                             wo_t[1][:, nr],
                                             start=False, stop=True)
                            fin = osb_pool.tile([128, 512], dt.float16,
                                                tag="fin")
                            if (tb + nch) % 2 == 0:
                                nc.scalar.activation(fin, ps, Act.Copy)
                            else:
                                nc.vector.tensor_copy(fin, ps)
                            nc.sync.dma_start(
                                out=out.ap()[b * S + tb * 128:
                                             b * S + (tb + 1) * 128, nr],
                                in_=fin)

    nc.compile()
    return nc


def make_in_maps(hidden_states, Wq, bq, Wk, bk, Wv, bv, Wo, bo):
    x2 = np.asarray(hidden_states, dtype=np.float32).reshape(T, D)
    xTr = np.ascontiguousarray(x2.T).astype(np.float16)
    scale = 1.0 / math.sqrt(HD)
    mask_np = np.triu(np.ones((128, 128), dtype=np.float16))
    ones_np = np.ones((128, 1), dtype=np.float16)
    in_maps = []
    for c in range(NCORES):
        sl = slice(c * DC, (c + 1) * DC)
        in_maps.append({
            "xT": xTr,
            "wq": np.ascontiguousarray((np.asarray(Wq)[sl] * scale).T).astype(np.float16),
            "wk": np.ascontiguousarray(np.asarray(Wk)[sl].T).astype(np.float16),
            "wv": np.ascontiguousarray(np.asarray(Wv)[sl].T).astype(np.float16),
            "wo": np.ascontiguousarray(np.asarray(Wo)[:, sl].T).astype(np.float16),
            "bq": np.ascontiguousarray(
                (np.asarray(bq)[sl] * scale).reshape(HC, 128)).astype(np.float32),
            "bk": np.ascontiguousarray(
                np.asarray(bk)[sl].reshape(HC, 128)).astype(np.float32),
            "bv": np.ascontiguousarray(
                np.asarray(bv)[sl].reshape(1, DC)).astype(np.float32),
            "mask": mask_np,
            "ones": ones_np,
        })
    return in_maps


def kernel(hidden_states, Wq, bq, Wk, bk, Wv, bv, Wo, bo):
    if "nc" not in _CACHE:
        _CACHE["nc"] = build_nc()
    nc = _CACHE["nc"]
    in_maps = make_in_maps(hidden_states, Wq, bq, Wk, bk, Wv, bv, Wo, bo)
    res = run_bass_kernel_spmd(nc, in_maps, list(range(NCORES)))
    acc = np.zeros((T, D), dtype=np.float64)
    for c in range(NCORES):
        acc += res.results[c]["out"].astype(np.float64)
    acc += np.asarray(bo, dtype=np.float64)
    return acc.astype(np.float32).reshape(B, S, D)
